# revision 1
# baseline (speedup 1.0000x reference)
"""DeltaNet decode step on 8 Trainium2 NeuronCores (tensor-parallel over heads).

Contract: kernel(**inputs) takes the FULL unsharded inputs (numpy arrays,
same keys as the reference setup_inputs()) and returns the FULL output
[1, 4096, 1, 1] float32.

Sharding (8 cores, 16 heads -> 2 heads/core):
  - Wq/Wk rows, q/k conv weights+caches: 512 rows per core
  - Wv rows, v conv weights+caches, Wo columns: 1024 per core
  - state: 2 heads per core
  - output: each core computes a partial [4096] projection; host all-reduces.

Device kernel: memory-bound mat-vec streaming. Weights are host-transposed
(contraction dim on partitions) and split into bf16 hi/lo pairs; each fp32
matvec becomes 3 bf16 matmuls (hi*hhi + hi*hlo + lo*hhi) accumulated in
fp32 PSUM -> ~2^-16 relative error at 1 cycle/row on the PE.

The post-matvec chain (v-conv, state update, combine) runs in 128-lane
column layout so it stays off the DMA critical path; row->column moves use
K=1 outer-product matmuls (lhsT=[1,128] row slice, rhs=[1,1] const 1.0).
"""

import os
import sys
import types

sys.path.insert(0, "/opt/trn_rl_repo")

import numpy as np
import ml_dtypes

import concourse.bass as bass
import concourse.mybir as mybir
import concourse.tile as tile
from concourse import bacc
from concourse.bass_utils import run_bass_kernel_spmd

BF16 = ml_dtypes.bfloat16
F32 = mybir.dt.float32
BF = mybir.dt.bfloat16
AF = mybir.ActivationFunctionType
OP = mybir.AluOpType

H = 4096
QK = 4096
VD = 8192
EPS = 1e-6
NCORES = 8
HPC = 2          # heads per core
RQ = 512         # q/k rows per core
RV = 1024        # v rows / Wo cols per core

_CACHE = {}


def _ensure_ntff_hook():
    """Install the axon NTFF profile hook shim (antenv.axon_hooks is absent
    in this image). Harmless if profiling is never requested."""
    if "antenv.axon_hooks" in sys.modules:
        return
    try:
        import antenv
        mod = types.ModuleType("antenv.axon_hooks")
        mod._hook = None
        mod.set_axon_ntff_profile_hook = lambda h: setattr(mod, "_hook", h)
        mod.get_axon_ntff_profile_hook = lambda: mod._hook
        sys.modules["antenv.axon_hooks"] = mod
        antenv.axon_hooks = mod
        from trn_agent_boot.trn_boot import _ntff_profile_via_ctypes
        mod._hook = _ntff_profile_via_ctypes("/opt/axon/libaxon_pjrt.so")
    except Exception:
        pass


def _build_nc():
    nc = bacc.Bacc(None)

    d = {}
    d["wqkt_hi"] = nc.dram_tensor("wqkt_hi", [QK, 2 * RQ], BF, kind="ExternalInput")
    d["wqkt_lo"] = nc.dram_tensor("wqkt_lo", [QK, 2 * RQ], BF, kind="ExternalInput")
    d["wvt_hi"] = nc.dram_tensor("wvt_hi", [H, RV], BF, kind="ExternalInput")
    d["wvt_lo"] = nc.dram_tensor("wvt_lo", [H, RV], BF, kind="ExternalInput")
    d["wot_hi"] = nc.dram_tensor("wot_hi", [RV, H], BF, kind="ExternalInput")
    d["wot_lo"] = nc.dram_tensor("wot_lo", [RV, H], BF, kind="ExternalInput")
    d["wab"] = nc.dram_tensor("wab", [128, 128], F32, kind="ExternalInput")
    d["state_c"] = nc.dram_tensor("state_c", [128, 2048], F32, kind="ExternalInput")
    d["hb"] = nc.dram_tensor("hb", [128, 64], BF, kind="ExternalInput")
    d["h_f32"] = nc.dram_tensor("h_f32", [128, 32], F32, kind="ExternalInput")
    d["qkcache"] = nc.dram_tensor("qkcache", [128, 24], F32, kind="ExternalInput")
    d["qkconvw"] = nc.dram_tensor("qkconvw", [128, 32], F32, kind="ExternalInput")
    d["vcache"] = nc.dram_tensor("vcache", [128, 24], F32, kind="ExternalInput")
    d["vconvw"] = nc.dram_tensor("vconvw", [128, 32], F32, kind="ExternalInput")
    out_d = nc.dram_tensor("out", [1, H], F32, kind="ExternalOutput")

    with tile.TileContext(nc) as tc:
        with (
            tc.tile_pool(name="smalls", bufs=1) as sm,
            tc.tile_pool(name="wpool", bufs=8) as wp,
            tc.tile_pool(name="psum", bufs=8, space="PSUM") as pm,
        ):
            def emit():
                # ---- small input DMAs (SWDGE keeps the HWDGE rings clear) ----
                hb = sm.tile([128, 64], BF, tag="hb")
                hf = sm.tile([128, 32], F32, tag="hf")
                wab = sm.tile([128, 128], F32, tag="wab")
                st = sm.tile([128, 2048], F32, tag="st")
                qkca = sm.tile([128, 24], F32, tag="qkca")
                qkcw = sm.tile([128, 32], F32, tag="qkcw")
                vca = sm.tile([128, 24], F32, tag="vca")
                vcw = sm.tile([128, 32], F32, tag="vcw")
                for t, src in [(hb, "hb"), (hf, "h_f32"),
                               (wab, "wab"), (st, "state_c"),
                               (qkca, "qkcache"), (qkcw, "qkconvw"),
                               (vca, "vcache"), (vcw, "vconvw")]:
                    nc.gpsimd.dma_start(out=t[:], in_=d[src][:])
                ones = sm.tile([1, 128], F32, tag="ones")
                nc.vector.memset(ones[:], 1.0)
                ones2 = sm.tile([2, 1], F32, tag="ones2")
                nc.vector.memset(ones2[:], 1.0)
                onesc = sm.tile([128, 1], F32, tag="onesc")
                nc.vector.memset(onesc[:], 1.0)
                epst = sm.tile([1, 1], F32, tag="epst")
                nc.vector.memset(epst[:], EPS)

                # ---- alpha/beta matvec (fp32, tiny) ----
                ps_ab = pm.tile([1, 4], F32, tag="ps")
                for cc in range(32):
                    nc.tensor.matmul(
                        ps_ab[0:1, 0:4], hf[:, cc:cc + 1],
                        wab[:, 4 * cc:4 * cc + 4],
                        start=(cc == 0), stop=(cc == 31))
                ab = sm.tile([1, 4], F32, tag="ab")
                nc.scalar.activation(ab[:], ps_ab[:], AF.Sigmoid)

                # ---- big streaming matvecs ----
                def stream_tile(view, dsel, r_width):
                    t = wp.tile([128, 8192], BF, tag="w", name="wtile")
                    nc.sync.dma_start(
                        out=t[:].rearrange("p (i r) -> p i r", r=r_width),
                        in_=view[dsel])
                    return t

                def matvec(ps_list, hi_dram, lo_dram, r_width, use_m2=False,
                           inject=None):
                    """use_m2=True: accumulate into [2,512] psum tiles:
                    row0 += Whi.T@hhi + Wlo.T@hhi ; row1 += Whi.T@hlo, with
                    the two hi-tile terms fused in one M=2 matmul (fold rows
                    afterwards). use_m2=False: plain 3 M=1 matmuls per chunk
                    into row 0. inject: {dd: callable} emitted after dd."""
                    ipd = 8192 // r_width
                    n_d = 32 // ipd
                    nt = r_width // 512
                    view_hi = hi_dram.rearrange("(d i p) r -> d p i r", i=ipd, p=128)
                    view_lo = lo_dram.rearrange("(d i p) r -> d p i r", i=ipd, p=128)
                    cnt = [0]
                    total = n_d * ipd * (2 if use_m2 else 3)

                    def emit_mm(t, i, cc, m2):
                        for it in range(nt):
                            sl = t[:, r_width * i + 512 * it:
                                   r_width * i + 512 * it + 512]
                            if m2:
                                nc.tensor.matmul(
                                    ps_list[it][0:2, :], hb[:, 2 * cc:2 * cc + 2],
                                    sl, start=(cnt[0] == 0),
                                    stop=(cnt[0] == total - 1))
                            else:
                                nc.tensor.matmul(
                                    ps_list[it][0:1, :], hb[:, 2 * cc:2 * cc + 1],
                                    sl, start=(cnt[0] == 0),
                                    stop=(cnt[0] == total - 1))
                        cnt[0] += 1

                    def emit_mm3(t, i, cc, second):
                        # two M=1 matmuls: lhsT hhi then (hi-tile only) hlo
                        for col in ([2 * cc, 2 * cc + 1] if second else [2 * cc]):
                            for it in range(nt):
                                nc.tensor.matmul(
                                    ps_list[it][0:1, :], hb[:, col:col + 1],
                                    t[:, r_width * i + 512 * it:
                                      r_width * i + 512 * it + 512],
                                    start=(cnt[0] == 0),
                                    stop=(cnt[0] == total - 1))
                            cnt[0] += 1

                    for dd in range(n_d):
                        t_hi = stream_tile(view_hi, dd, r_width)
                        t_lo = stream_tile(view_lo, dd, r_width)
                        if not use_m2:
                            for i in range(ipd):
                                cc = ipd * dd + i
                                emit_mm3(t_hi, i, cc, True)
                                emit_mm3(t_lo, i, cc, False)
                        elif dd < n_d - 1:
                            for i in range(ipd):
                                cc = ipd * dd + i
                                emit_mm(t_hi, i, cc, True)
                                emit_mm(t_lo, i, cc, False)
                        else:
                            # last tile pair: lo (row0-only) first, hi (M=2)
                            # last so the closing stop covers both rows
                            for i in range(ipd):
                                emit_mm(t_lo, i, ipd * dd + i, False)
                            for i in range(ipd):
                                emit_mm(t_hi, i, ipd * dd + i, True)
                        if inject and dd in inject:
                            inject[dd]()

                # q and k matvecs fused: rhs chunks are [Wq.T | Wk.T] packed.
                ps_q = pm.tile([1, 512], F32, tag="ps")
                ps_k = pm.tile([1, 512], F32, tag="ps")
                matvec([ps_q, ps_k], d["wqkt_hi"], d["wqkt_lo"], 1024)
                qrow = sm.tile([1, 512], F32, tag="qrow")
                nc.vector.tensor_copy(qrow[:], ps_q[0:1, :])
                krow = sm.tile([1, 512], F32, tag="krow")
                nc.scalar.copy(krow[:], ps_k[0:1, :])

                # The rest of the q/k chain runs in 128-lane column layout
                # (cols 0-3 = k chunks, 4-7 = q chunks); the per-head
                # reductions (l2norm sum-sq, q.k dot) use ones-column fp32
                # matmuls for the partition-dim sum. All PE pieces are
                # injected into the Wv streaming phase to fill DMA-wait gaps.
                t_qk = pm.tile([128, 8], F32, tag="ps")
                qkcol = sm.tile([128, 8], F32, tag="qkcol")
                qacc = sm.tile([128, 8], F32, tag="qacc")
                qtmp = sm.tile([128, 8], F32, tag="qtmp")
                x1 = sm.tile([128, 8], F32, tag="x1")
                sq = sm.tile([128, 8], F32, tag="sq")
                ps_ss = pm.tile([1, 8], F32, tag="ps")
                ssr = sm.tile([1, 8], F32, tag="ssr")
                ssh = sm.tile([1, 4], F32, tag="ssh")
                srt = sm.tile([1, 4], F32, tag="srt")
                rin = sm.tile([1, 4], F32, tag="rin")
                t_rn = pm.tile([128, 4], F32, tag="ps")
                rbc = sm.tile([128, 4], F32, tag="rbc")
                qkn = sm.tile([128, 8], F32, tag="qkn")
                dm = sm.tile([128, 4], F32, tag="dm")
                ps_dot = pm.tile([1, 4], F32, tag="ps")
                dotr = sm.tile([1, 4], F32, tag="dotr")
                dot = sm.tile([1, 2], F32, tag="dot")
                bd = sm.tile([1, 2], F32, tag="bd")
                t_bc = pm.tile([128, 4], F32, tag="ps")
                abc = sm.tile([128, 4], F32, tag="abc")
                ps_stc = pm.tile([128, 16], F32, tag="ps")

                def chain_pe_0():
                    # raw q/k rows -> columns (K=1 outer products)
                    for c in range(4):
                        nc.tensor.matmul(t_qk[:, c:c + 1],
                                         krow[0:1, 128 * c:128 * c + 128],
                                         ones[0:1, 0:1], start=True, stop=True)
                        nc.tensor.matmul(t_qk[:, 4 + c:5 + c],
                                         qrow[0:1, 128 * c:128 * c + 128],
                                         ones[0:1, 0:1], start=True, stop=True)
                    nc.vector.tensor_copy(qkcol[:], t_qk[:])
                    # conv + silu in columns
                    nc.vector.tensor_mul(qacc[:], qkca[:, 0:8], qkcw[:, 0:8])
                    for tpi in (1, 2):
                        nc.vector.tensor_mul(qtmp[:], qkca[:, 8 * tpi:8 * tpi + 8],
                                             qkcw[:, 8 * tpi:8 * tpi + 8])
                        nc.vector.tensor_add(qacc[:], qacc[:], qtmp[:])
                    nc.vector.tensor_mul(qtmp[:], qkcol[:], qkcw[:, 24:32])
                    nc.vector.tensor_add(qacc[:], qacc[:], qtmp[:])
                    nc.scalar.activation(x1[:], qacc[:], AF.Sigmoid)
                    nc.vector.tensor_mul(x1[:], qacc[:], x1[:])
                    nc.vector.tensor_mul(sq[:], x1[:], x1[:])

                def chain_pe_1():
                    # per-column sum of squares, then per-head l2 scale
                    nc.tensor.matmul(ps_ss[0:1, :], onesc[:, 0:1], sq[:],
                                     start=True, stop=True)
                    nc.vector.tensor_copy(ssr[:], ps_ss[0:1, :])
                    nc.vector.reduce_sum(
                        ssh[0:1, 0:4],
                        ssr[0:1, :].rearrange("a (g t) -> a g t", t=2),
                        axis=mybir.AxisListType.X)
                    nc.scalar.activation(srt[:], ssh[:], AF.Sqrt,
                                         bias=epst[0:1, 0:1])
                    nc.vector.reciprocal(rin[:], srt[:])

                def chain_pe_2():
                    # broadcast 1/norm, normalize columns
                    for j in range(4):
                        nc.tensor.matmul(t_rn[:, j:j + 1], ones[0:1, :],
                                         rin[0:1, j:j + 1], start=True, stop=True)
                    nc.vector.tensor_copy(rbc[:], t_rn[:])
                    for g in range(4):  # k_h0, k_h1, q_h0, q_h1 col pairs
                        nc.vector.tensor_scalar(
                            out=qkn[:, 2 * g:2 * g + 2],
                            in0=x1[:, 2 * g:2 * g + 2],
                            scalar1=rbc[:, g:g + 1], scalar2=None, op0=OP.mult)
                    # q.k dot per head
                    nc.vector.tensor_mul(dm[:], qkn[:, 4:8], qkn[:, 0:4])
                    nc.tensor.matmul(ps_dot[0:1, :], onesc[:, 0:1], dm[:],
                                     start=True, stop=True)
                    nc.vector.tensor_copy(dotr[:], ps_dot[0:1, :])
                    nc.vector.reduce_sum(
                        dot[0:1, 0:2],
                        dotr[0:1, :].rearrange("a (g t) -> a g t", t=2),
                        axis=mybir.AxisListType.X)
                    nc.vector.tensor_mul(bd[:], ab[0:1, 2:4], dot[0:1, 0:2])
                    # broadcast alpha / beta*dot to partitions
                    for hh in range(HPC):
                        nc.tensor.matmul(t_bc[:, hh:hh + 1], ones[0:1, :],
                                         ab[0:1, hh:hh + 1],
                                         start=True, stop=True)
                        nc.tensor.matmul(t_bc[:, 2 + hh:3 + hh], ones[0:1, :],
                                         bd[0:1, hh:hh + 1],
                                         start=True, stop=True)
                    nc.vector.tensor_copy(abc[:], t_bc[:])
                    # state matvecs (fp32, column outputs)
                    for hh in range(HPC):
                        for which in range(2):  # 0 -> k, 1 -> q
                            for vc in range(4):
                                col = 8 * which + 4 * hh + vc
                                for d2 in range(2):
                                    blk = 2 * hh + d2
                                    nc.tensor.matmul(
                                        ps_stc[:, col:col + 1],
                                        st[:, 512 * blk + 128 * vc:
                                           512 * blk + 128 * vc + 128],
                                        qkn[:, 4 * which + 2 * hh + d2:
                                            4 * which + 2 * hh + d2 + 1],
                                        start=(d2 == 0), stop=(d2 == 1))

                # ---- v matvec (rows x2 psum rows), fold+transpose to cols ----
                ps_v0 = pm.tile([2, 512], F32, tag="ps")
                ps_v1 = pm.tile([2, 512], F32, tag="ps")
                matvec([ps_v0, ps_v1], d["wvt_hi"], d["wvt_lo"], 1024,
                       use_m2=True,
                       inject={0: chain_pe_0, 1: chain_pe_1, 2: chain_pe_2})
                vsb = sm.tile([2, 1024], F32, tag="vsb")
                nc.vector.tensor_copy(vsb[0:2, 0:512], ps_v0[:])
                nc.scalar.copy(vsb[0:2, 512:1024], ps_v1[:])
                # K=2 transpose-fold: vcol[p, j] = vsb[0,128j+p] + vsb[1,128j+p]
                t_v = pm.tile([128, 8], F32, tag="ps")
                for j in range(8):
                    nc.tensor.matmul(t_v[:, j:j + 1],
                                     vsb[0:2, 128 * j:128 * j + 128],
                                     ones2[0:2, 0:1], start=True, stop=True)
                vcol = sm.tile([128, 8], F32, tag="vcol")
                nc.vector.tensor_copy(vcol[:], t_v[:])

                # ---- v conv + silu in columns [128, 8] ----
                vacc = sm.tile([128, 8], F32, tag="vacc")
                vtmp = sm.tile([128, 8], F32, tag="vtmp")
                nc.vector.tensor_mul(vacc[:], vca[:, 0:8], vcw[:, 0:8])
                for tpi in (1, 2):
                    nc.vector.tensor_mul(vtmp[:], vca[:, 8 * tpi:8 * tpi + 8],
                                         vcw[:, 8 * tpi:8 * tpi + 8])
                    nc.vector.tensor_add(vacc[:], vacc[:], vtmp[:])
                nc.vector.tensor_mul(vtmp[:], vcol[:], vcw[:, 24:32])
                nc.vector.tensor_add(vacc[:], vacc[:], vtmp[:])
                v1c = sm.tile([128, 8], F32, tag="v1c")
                nc.scalar.activation(v1c[:], vacc[:], AF.Sigmoid)
                nc.vector.tensor_mul(v1c[:], vacc[:], v1c[:])

                # ---- combine in columns: ov = a*qs + (b*dot)*(v - a*ks) ----
                ovc = sm.tile([128, 8], F32, tag="ovc")
                errc = sm.tile([128, 4], F32, tag="errc")
                t1c = sm.tile([128, 4], F32, tag="t1c")
                for hh in range(HPC):
                    ks = ps_stc[:, 4 * hh:4 * hh + 4]
                    qs = ps_stc[:, 8 + 4 * hh:8 + 4 * hh + 4]
                    nc.vector.tensor_scalar(out=errc[:], in0=ks,
                                            scalar1=abc[:, hh:hh + 1],
                                            scalar2=None, op0=OP.mult)
                    nc.vector.tensor_sub(errc[:], v1c[:, 4 * hh:4 * hh + 4], errc[:])
                    nc.vector.tensor_scalar(out=t1c[:], in0=qs,
                                            scalar1=abc[:, hh:hh + 1],
                                            scalar2=None, op0=OP.mult)
                    nc.vector.tensor_scalar(out=errc[:], in0=errc[:],
                                            scalar1=abc[:, 2 + hh:3 + hh],
                                            scalar2=None, op0=OP.mult)
                    nc.vector.tensor_add(ovc[:, 4 * hh:4 * hh + 4], t1c[:], errc[:])

                # ---- split ov to bf16 hi/lo columns ----
                ov_hi = sm.tile([128, 8], BF, tag="ov_hi")
                nc.vector.tensor_copy(ov_hi[:], ovc[:])
                ov_hi32 = sm.tile([128, 8], F32, tag="ov_hi32")
                nc.vector.tensor_copy(ov_hi32[:], ov_hi[:])
                ov_lo32 = sm.tile([128, 8], F32, tag="ov_lo32")
                nc.vector.tensor_sub(ov_lo32[:], ovc[:], ov_hi32[:])
                ov_lo = sm.tile([128, 8], BF, tag="ov_lo")
                nc.vector.tensor_copy(ov_lo[:], ov_lo32[:])

                # ---- output projection ----
                ps_o = [pm.tile([1, 512], F32, tag="ps", name=f"ps_o{i}")
                        for i in range(8)]
                view_ohi = d["wot_hi"].rearrange("(d i p) r -> d p i r", i=2, p=128)
                view_olo = d["wot_lo"].rearrange("(d i p) r -> d p i r", i=2, p=128)
                out_sb = sm.tile([1, H], F32, tag="out_sb")
                for dd in range(4):
                    t_hi = stream_tile(view_ohi, dd, 4096)
                    t_lo = stream_tile(view_olo, dd, 4096)
                    for i in range(2):
                        j = 2 * dd + i
                        for it in range(8):
                            sl = slice(4096 * i + 512 * it,
                                       4096 * i + 512 * it + 512)
                            nc.tensor.matmul(ps_o[it][0:1, :], ov_hi[:, j:j + 1],
                                             t_hi[:, sl], start=(j == 0), stop=False)
                            nc.tensor.matmul(ps_o[it][0:1, :], ov_lo[:, j:j + 1],
                                             t_hi[:, sl], start=False, stop=False)
                            nc.tensor.matmul(ps_o[it][0:1, :], ov_hi[:, j:j + 1],
                                             t_lo[:, sl], start=False, stop=(j == 7))
                for it in range(8):
                    dst = out_sb[0:1, 512 * it:512 * it + 512]
                    if it % 2 == 0:
                        nc.vector.tensor_copy(dst, ps_o[it][0:1, :])
                    else:
                        nc.scalar.copy(dst, ps_o[it][0:1, :])
                nc.sync.dma_start(out=out_d[:], in_=out_sb[:])

            emit()

    nc.finalize()
    return nc


def _split_bf16_T(m):
    """m [R, C] f32 -> (hi.T, lo.T) contiguous [C, R] bf16."""
    hi = m.astype(BF16)
    lo = (m - hi.astype(np.float32)).astype(BF16)
    return np.ascontiguousarray(hi.T), np.ascontiguousarray(lo.T)


def _prep_in_maps(inputs):
    f32 = np.float32
    hid = np.asarray(inputs["hidden_states"], f32)[0, :, 0, 0]     # [4096]
    Wq = np.asarray(inputs["Wq"], f32)
    Wk = np.asarray(inputs["Wk"], f32)
    Wv = np.asarray(inputs["Wv"], f32)
    Wo = np.asarray(inputs["Wo"], f32)
    Wa = np.asarray(inputs["Wa"], f32)
    Wb = np.asarray(inputs["Wb"], f32)
    qcw = np.asarray(inputs["q_conv_w"], f32)[0]                   # [QK, 4]
    kcw = np.asarray(inputs["k_conv_w"], f32)[0]
    vcw = np.asarray(inputs["v_conv_w"], f32)[0]                   # [VD, 4]
    qca = np.asarray(inputs["q_cache"], f32)[0]                    # [QK, 3]
    kca = np.asarray(inputs["k_cache"], f32)[0]
    vca = np.asarray(inputs["v_cache"], f32)[0]                    # [VD, 3]
    state = np.asarray(inputs["state"], f32)[0]                    # [16,256,512]

    h_hi = hid.astype(BF16)
    h_lo = (hid - h_hi.astype(f32)).astype(BF16)
    cols = lambda v: np.ascontiguousarray(v.reshape(32, 128).T)
    h_hi_c, h_lo_c, h_f_c = cols(h_hi), cols(h_lo), cols(hid)
    hb_c = np.ascontiguousarray(
        np.stack([h_hi_c, h_lo_c], axis=2).reshape(128, 64))

    in_maps = []
    for c in range(NCORES):
        rq = slice(c * RQ, (c + 1) * RQ)
        rv = slice(c * RV, (c + 1) * RV)
        # packed [Wq ; Wk] rows -> transposed [H, 1024]
        wqk = np.concatenate([Wq[rq], Wk[rq]], axis=0)             # [1024, 4096]
        wqkt_hi, wqkt_lo = _split_bf16_T(wqk)
        wvt_hi, wvt_lo = _split_bf16_T(Wv[rv])
        wot_hi, wot_lo = _split_bf16_T(Wo[:, rv])                  # [RV, H]

        wab = np.concatenate([Wa[2 * c:2 * c + 2], Wb[2 * c:2 * c + 2]], 0)
        wab_sb = np.ascontiguousarray(
            wab.reshape(4, 32, 128).transpose(2, 1, 0).reshape(128, 128))
        st_sb = np.ascontiguousarray(
            state[2 * c:2 * c + 2].reshape(2, 2, 128, 512)
            .transpose(2, 0, 1, 3).reshape(128, 2048))

        # q/k conv in column layout [128, 8*taps]: per tap, cols 0-3 = k
        # chunks (k idx 128c+p), cols 4-7 = q chunks
        qk_ca = np.concatenate(
            [np.concatenate([kca[rq, t].reshape(4, 128).T,
                             qca[rq, t].reshape(4, 128).T], 1)
             for t in range(3)], 1)
        qk_cw = np.concatenate(
            [np.concatenate([kcw[rq, t].reshape(4, 128).T,
                             qcw[rq, t].reshape(4, 128).T], 1)
             for t in range(4)], 1)
        # v conv in column layout [128, 8*taps]: vcol[p, 8t+cc] = v[128cc+p, t]
        v_ca = np.ascontiguousarray(
            vca[rv].reshape(8, 128, 3).transpose(1, 2, 0).reshape(128, 24))
        v_cw = np.ascontiguousarray(
            vcw[rv].reshape(8, 128, 4).transpose(1, 2, 0).reshape(128, 32))

        in_maps.append({
            "wqkt_hi": wqkt_hi, "wqkt_lo": wqkt_lo,
            "wvt_hi": wvt_hi, "wvt_lo": wvt_lo,
            "wot_hi": wot_hi, "wot_lo": wot_lo,
            "wab": wab_sb, "state_c": st_sb,
            "hb": hb_c, "h_f32": h_f_c,
            "qkcache": np.ascontiguousarray(qk_ca),
            "qkconvw": np.ascontiguousarray(qk_cw),
            "vcache": v_ca, "vconvw": v_cw,
        })
    return in_maps


def _run(inputs, trace=False, tmpdir=None):
    _ensure_ntff_hook()
    if "nc" not in _CACHE:
        _CACHE["nc"] = _build_nc()
    nc = _CACHE["nc"]
    in_maps = _prep_in_maps(inputs)
    res = run_bass_kernel_spmd(nc, in_maps, list(range(NCORES)),
                               trace=trace, tmpdir=tmpdir)
    acc = np.zeros(H, np.float64)
    for c in range(NCORES):
        acc += res.results[c]["out"][0].astype(np.float64)
    out = acc.astype(np.float32).reshape(1, H, 1, 1)
    return out, res


def kernel(**inputs):
    out, _ = _run(inputs, trace=False)
    return out


def kernel_traced(tmpdir=None, **inputs):
    return _run(inputs, trace=True, tmpdir=tmpdir)



# revision 5
# speedup vs baseline: 2.1315x; 2.1315x over previous
"""DeltaNet decode step on 8 Trainium2 NeuronCores (tensor-parallel over heads).

Contract: kernel(**inputs) takes the FULL unsharded inputs (numpy arrays,
same keys as the reference setup_inputs()) and returns the FULL output
[1, 4096, 1, 1] float32.

Sharding (8 cores, 16 heads -> 2 heads/core):
  - Wq/Wk rows, q/k conv weights+caches: 512 rows per core
  - Wv rows, v conv weights+caches, Wo columns: 1024 per core
  - state: 2 heads per core
  - output: each core computes a partial [4096] projection; host all-reduces.

Device kernel: memory-bound mat-vec streaming. Weights are quantized to
single bf16 (the end-to-end rel-err of the all-bf16 pipeline is ~3e-3,
well inside the 2e-2 gate) and pre-packed on host into one contiguous
DRAM image per core, laid out exactly as the SBUF tiles consume them:
12 tiles of [128, 8192] bf16 (2 MB each) — 4 for [Wq|Wk].T, 4 for Wv.T,
4 for Wo.T. Each fp32 matvec is a single bf16 matmul per 512-column
slice accumulated in fp32 PSUM.

The post-matvec chain (conv+silu, l2norm, state update, combine) runs in
128-lane column layout injected into the Wv streaming phase so it stays
off the DMA critical path; row->column moves use K=1 outer-product
matmuls (lhsT=[1,128] row slice, rhs=[1,1] const 1.0).
"""

import os
import sys
import types

sys.path.insert(0, "/opt/trn_rl_repo")

import numpy as np
import ml_dtypes

import concourse.bass as bass
import concourse.mybir as mybir
import concourse.tile as tile
from concourse import bacc
from concourse.bass_utils import run_bass_kernel_spmd

BF16 = ml_dtypes.bfloat16
F32 = mybir.dt.float32
BF = mybir.dt.bfloat16
AF = mybir.ActivationFunctionType
OP = mybir.AluOpType

H = 4096
QK = 4096
VD = 8192
EPS = 1e-6
NCORES = 8
HPC = 2          # heads per core
RQ = 512         # q/k rows per core
RV = 1024        # v rows / Wo cols per core
NTILES = 12      # big weight tiles per core: 4 qk + 4 v + 4 o

_CACHE = {}


def _ensure_ntff_hook():
    """Install the axon NTFF profile hook shim (antenv.axon_hooks is absent
    in this image). Harmless if profiling is never requested."""
    if "antenv.axon_hooks" in sys.modules:
        return
    try:
        import antenv
        mod = types.ModuleType("antenv.axon_hooks")
        mod._hook = None
        mod.set_axon_ntff_profile_hook = lambda h: setattr(mod, "_hook", h)
        mod.get_axon_ntff_profile_hook = lambda: mod._hook
        sys.modules["antenv.axon_hooks"] = mod
        antenv.axon_hooks = mod
        from trn_agent_boot.trn_boot import _ntff_profile_via_ctypes
        mod._hook = _ntff_profile_via_ctypes("/opt/axon/libaxon_pjrt.so")
    except Exception:
        pass


def _build_nc():
    nc = bacc.Bacc(None)

    d = {}
    d["wbig"] = nc.dram_tensor("wbig", [NTILES * 128, 8192], BF, kind="ExternalInput")
    d["hb"] = nc.dram_tensor("hb", [128, 32], BF, kind="ExternalInput")
    d["wabt"] = nc.dram_tensor("wabt", [128, 128], BF, kind="ExternalInput")
    d["state_c"] = nc.dram_tensor("state_c", [128, 2048], BF, kind="ExternalInput")
    d["qkcache"] = nc.dram_tensor("qkcache", [128, 24], F32, kind="ExternalInput")
    d["qkconvw"] = nc.dram_tensor("qkconvw", [128, 32], F32, kind="ExternalInput")
    d["vcache"] = nc.dram_tensor("vcache", [128, 24], F32, kind="ExternalInput")
    d["vconvw"] = nc.dram_tensor("vconvw", [128, 32], F32, kind="ExternalInput")
    out_d = nc.dram_tensor("out", [1, H], F32, kind="ExternalOutput")

    with tile.TileContext(nc) as tc:
        with (
            tc.tile_pool(name="smalls", bufs=1) as sm,
            tc.tile_pool(name="wpool", bufs=8) as wp,
            tc.tile_pool(name="psum", bufs=8, space="PSUM") as pm,
        ):
            def emit():
                # ---- small input DMAs (SWDGE keeps the HWDGE ring clear) ----
                hb = sm.tile([128, 32], BF, tag="hb")
                wab = sm.tile([128, 128], BF, tag="wab")
                st = sm.tile([128, 2048], BF, tag="st")
                qkca = sm.tile([128, 24], F32, tag="qkca")
                qkcw = sm.tile([128, 32], F32, tag="qkcw")
                vca = sm.tile([128, 24], F32, tag="vca")
                vcw = sm.tile([128, 32], F32, tag="vcw")
                for t, src in [(hb, "hb"), (wab, "wabt"), (st, "state_c"),
                               (qkca, "qkcache"), (qkcw, "qkconvw"),
                               (vca, "vcache"), (vcw, "vconvw")]:
                    nc.gpsimd.dma_start(out=t[:], in_=d[src][:])
                ones = sm.tile([1, 128], F32, tag="ones")
                nc.vector.memset(ones[:], 1.0)
                onesc = sm.tile([128, 1], F32, tag="onesc")
                nc.vector.memset(onesc[:], 1.0)
                epst = sm.tile([1, 1], F32, tag="epst")
                nc.vector.memset(epst[:], EPS)

                # ---- big weight tile stream, 2 HWDGE rings ----
                # 13 DMAs: t0-3 qk, t4-7 v, t8-10 wo (2MB), t11a/t11b (1MB).
                # Issue index alternates sync/scalar rings so per-instruction
                # completion bubbles on one ring overlap the other's stream.
                # First 8 are pre-issued (fills the 8-buffer pool); the rest
                # are issued right after an early tile's matmuls so the ring
                # FIFO never head-of-line blocks and the ACT ring's DMA
                # issues all precede its chain-compute instructions.
                wview = d["wbig"].rearrange("(d p) r -> d p r", p=128)
                rings = [nc.sync, nc.scalar]
                tiles = []
                issue_cnt = [0]

                def issue_tile(src_ap, width):
                    t = wp.tile([128, width], BF, tag="w", name="wtile")
                    rings[issue_cnt[0] % 2].dma_start(out=t[:], in_=src_ap)
                    issue_cnt[0] += 1
                    tiles.append(t)

                for idx in range(8):
                    issue_tile(wview[idx], 8192)
                late = [lambda: issue_tile(wview[8], 8192),
                        lambda: issue_tile(wview[9], 8192),
                        lambda: issue_tile(wview[10], 8192),
                        lambda: issue_tile(wview[11][:, 0:4096], 4096),
                        lambda: issue_tile(wview[11][:, 4096:8192], 4096)]

                # ---- q/k matvec: rhs chunks are [Wq.T | Wk.T] packed ----
                ps_q = pm.tile([1, 512], F32, tag="ps")
                ps_k = pm.tile([1, 512], F32, tag="ps")
                for dd in range(4):
                    t = tiles[dd]
                    for i in range(8):
                        cc = 8 * dd + i
                        nc.tensor.matmul(
                            ps_q[0:1, :], hb[:, cc:cc + 1],
                            t[:, 1024 * i:1024 * i + 512],
                            start=(cc == 0), stop=(cc == 31))
                        nc.tensor.matmul(
                            ps_k[0:1, :], hb[:, cc:cc + 1],
                            t[:, 1024 * i + 512:1024 * i + 1024],
                            start=(cc == 0), stop=(cc == 31))
                    late[dd]()
                qrow = sm.tile([1, 512], F32, tag="qrow")
                nc.vector.tensor_copy(qrow[:], ps_q[0:1, :])
                krow = sm.tile([1, 512], F32, tag="krow")
                nc.scalar.copy(krow[:], ps_k[0:1, :])

                # ---- alpha/beta matvec (bf16, tiny) ----
                ps_ab = pm.tile([1, 4], F32, tag="ps")
                for cc in range(32):
                    nc.tensor.matmul(
                        ps_ab[0:1, 0:4], hb[:, cc:cc + 1],
                        wab[:, 4 * cc:4 * cc + 4],
                        start=(cc == 0), stop=(cc == 31))
                ab = sm.tile([1, 4], F32, tag="ab")
                nc.scalar.activation(ab[:], ps_ab[:], AF.Sigmoid)

                # The rest of the q/k chain runs in 128-lane column layout
                # (cols 0-3 = k chunks, 4-7 = q chunks); the per-head
                # reductions (l2norm sum-sq, q.k dot) use ones-column fp32
                # matmuls for the partition-dim sum. All PE pieces are
                # injected into the Wv streaming phase to fill DMA-wait gaps.
                t_qk = pm.tile([128, 8], F32, tag="ps")
                qkcol = sm.tile([128, 8], F32, tag="qkcol")
                qacc = sm.tile([128, 8], F32, tag="qacc")
                qtmp = sm.tile([128, 8], F32, tag="qtmp")
                x1 = sm.tile([128, 8], F32, tag="x1")
                sq = sm.tile([128, 8], F32, tag="sq")
                ps_ss = pm.tile([1, 8], F32, tag="ps")
                ssr = sm.tile([1, 8], F32, tag="ssr")
                ssh = sm.tile([1, 4], F32, tag="ssh")
                srt = sm.tile([1, 4], F32, tag="srt")
                rin = sm.tile([1, 4], F32, tag="rin")
                t_rn = pm.tile([128, 4], F32, tag="ps")
                rbc = sm.tile([128, 4], F32, tag="rbc")
                qkn = sm.tile([128, 8], F32, tag="qkn")
                qkn_b = sm.tile([128, 8], BF, tag="qkn_b")
                dm = sm.tile([128, 4], F32, tag="dm")
                ps_dot = pm.tile([1, 4], F32, tag="ps")
                dotr = sm.tile([1, 4], F32, tag="dotr")
                dot = sm.tile([1, 2], F32, tag="dot")
                bd = sm.tile([1, 2], F32, tag="bd")
                t_bc = pm.tile([128, 4], F32, tag="ps")
                abc = sm.tile([128, 4], F32, tag="abc")
                ps_stc = pm.tile([128, 16], F32, tag="ps")

                def chain_pe_0():
                    # raw q/k rows -> columns (K=1 outer products)
                    for c in range(4):
                        nc.tensor.matmul(t_qk[:, c:c + 1],
                                         krow[0:1, 128 * c:128 * c + 128],
                                         ones[0:1, 0:1], start=True, stop=True)
                        nc.tensor.matmul(t_qk[:, 4 + c:5 + c],
                                         qrow[0:1, 128 * c:128 * c + 128],
                                         ones[0:1, 0:1], start=True, stop=True)
                    nc.vector.tensor_copy(qkcol[:], t_qk[:])
                    # conv + silu in columns
                    nc.vector.tensor_mul(qacc[:], qkca[:, 0:8], qkcw[:, 0:8])
                    for tpi in (1, 2):
                        nc.vector.tensor_mul(qtmp[:], qkca[:, 8 * tpi:8 * tpi + 8],
                                             qkcw[:, 8 * tpi:8 * tpi + 8])
                        nc.vector.tensor_add(qacc[:], qacc[:], qtmp[:])
                    nc.vector.tensor_mul(qtmp[:], qkcol[:], qkcw[:, 24:32])
                    nc.vector.tensor_add(qacc[:], qacc[:], qtmp[:])
                    nc.scalar.activation(x1[:], qacc[:], AF.Sigmoid)
                    nc.vector.tensor_mul(x1[:], qacc[:], x1[:])
                    nc.vector.tensor_mul(sq[:], x1[:], x1[:])

                def chain_pe_1():
                    # per-column sum of squares, then per-head l2 scale
                    nc.tensor.matmul(ps_ss[0:1, :], onesc[:, 0:1], sq[:],
                                     start=True, stop=True)
                    nc.vector.tensor_copy(ssr[:], ps_ss[0:1, :])
                    nc.vector.reduce_sum(
                        ssh[0:1, 0:4],
                        ssr[0:1, :].rearrange("a (g t) -> a g t", t=2),
                        axis=mybir.AxisListType.X)
                    nc.scalar.activation(srt[:], ssh[:], AF.Sqrt,
                                         bias=epst[0:1, 0:1])
                    nc.vector.reciprocal(rin[:], srt[:])

                def chain_pe_2():
                    # broadcast 1/norm, normalize columns
                    for j in range(4):
                        nc.tensor.matmul(t_rn[:, j:j + 1], ones[0:1, :],
                                         rin[0:1, j:j + 1], start=True, stop=True)
                    nc.vector.tensor_copy(rbc[:], t_rn[:])
                    for g in range(4):  # k_h0, k_h1, q_h0, q_h1 col pairs
                        nc.vector.tensor_scalar(
                            out=qkn[:, 2 * g:2 * g + 2],
                            in0=x1[:, 2 * g:2 * g + 2],
                            scalar1=rbc[:, g:g + 1], scalar2=None, op0=OP.mult)
                    nc.vector.tensor_copy(qkn_b[:], qkn[:])
                    # q.k dot per head
                    nc.vector.tensor_mul(dm[:], qkn[:, 4:8], qkn[:, 0:4])
                    nc.tensor.matmul(ps_dot[0:1, :], onesc[:, 0:1], dm[:],
                                     start=True, stop=True)
                    nc.vector.tensor_copy(dotr[:], ps_dot[0:1, :])
                    nc.vector.reduce_sum(
                        dot[0:1, 0:2],
                        dotr[0:1, :].rearrange("a (g t) -> a g t", t=2),
                        axis=mybir.AxisListType.X)
                    nc.vector.tensor_mul(bd[:], ab[0:1, 2:4], dot[0:1, 0:2])
                    # broadcast alpha / beta*dot to partitions
                    for hh in range(HPC):
                        nc.tensor.matmul(t_bc[:, hh:hh + 1], ones[0:1, :],
                                         ab[0:1, hh:hh + 1],
                                         start=True, stop=True)
                        nc.tensor.matmul(t_bc[:, 2 + hh:3 + hh], ones[0:1, :],
                                         bd[0:1, hh:hh + 1],
                                         start=True, stop=True)
                    nc.vector.tensor_copy(abc[:], t_bc[:])
                    # state matvecs (bf16 state, column outputs)
                    for hh in range(HPC):
                        for which in range(2):  # 0 -> k, 1 -> q
                            for vc in range(4):
                                col = 8 * which + 4 * hh + vc
                                for d2 in range(2):
                                    blk = 2 * hh + d2
                                    nc.tensor.matmul(
                                        ps_stc[:, col:col + 1],
                                        st[:, 512 * blk + 128 * vc:
                                           512 * blk + 128 * vc + 128],
                                        qkn_b[:, 4 * which + 2 * hh + d2:
                                              4 * which + 2 * hh + d2 + 1],
                                        start=(d2 == 0), stop=(d2 == 1))

                # ---- v matvec, with chain injected into DMA-wait gaps ----
                ps_v0 = pm.tile([1, 512], F32, tag="ps")
                ps_v1 = pm.tile([1, 512], F32, tag="ps")
                inject = {0: chain_pe_0, 1: chain_pe_1, 2: chain_pe_2}
                for dd in range(4):
                    t = tiles[4 + dd]
                    for i in range(8):
                        cc = 8 * dd + i
                        nc.tensor.matmul(
                            ps_v0[0:1, :], hb[:, cc:cc + 1],
                            t[:, 1024 * i:1024 * i + 512],
                            start=(cc == 0), stop=(cc == 31))
                        nc.tensor.matmul(
                            ps_v1[0:1, :], hb[:, cc:cc + 1],
                            t[:, 1024 * i + 512:1024 * i + 1024],
                            start=(cc == 0), stop=(cc == 31))
                    if dd == 0:
                        late[4]()
                    if dd in inject:
                        inject[dd]()

                vsb = sm.tile([1, 1024], F32, tag="vsb")
                nc.vector.tensor_copy(vsb[0:1, 0:512], ps_v0[0:1, :])
                nc.scalar.copy(vsb[0:1, 512:1024], ps_v1[0:1, :])
                # transpose v row to columns: vcol[p, j] = vsb[0, 128j+p]
                t_v = pm.tile([128, 8], F32, tag="ps")
                for j in range(8):
                    nc.tensor.matmul(t_v[:, j:j + 1],
                                     vsb[0:1, 128 * j:128 * j + 128],
                                     ones[0:1, 0:1], start=True, stop=True)
                vcol = sm.tile([128, 8], F32, tag="vcol")
                nc.vector.tensor_copy(vcol[:], t_v[:])

                # ---- v conv + silu in columns [128, 8] ----
                vacc = sm.tile([128, 8], F32, tag="vacc")
                vtmp = sm.tile([128, 8], F32, tag="vtmp")
                nc.vector.tensor_mul(vacc[:], vca[:, 0:8], vcw[:, 0:8])
                for tpi in (1, 2):
                    nc.vector.tensor_mul(vtmp[:], vca[:, 8 * tpi:8 * tpi + 8],
                                         vcw[:, 8 * tpi:8 * tpi + 8])
                    nc.vector.tensor_add(vacc[:], vacc[:], vtmp[:])
                nc.vector.tensor_mul(vtmp[:], vcol[:], vcw[:, 24:32])
                nc.vector.tensor_add(vacc[:], vacc[:], vtmp[:])
                v1c = sm.tile([128, 8], F32, tag="v1c")
                nc.scalar.activation(v1c[:], vacc[:], AF.Sigmoid)
                nc.vector.tensor_mul(v1c[:], vacc[:], v1c[:])

                # ---- combine in columns: ov = a*qs + (b*dot)*(v - a*ks) ----
                ovc = sm.tile([128, 8], F32, tag="ovc")
                errc = sm.tile([128, 4], F32, tag="errc")
                t1c = sm.tile([128, 4], F32, tag="t1c")
                for hh in range(HPC):
                    ks = ps_stc[:, 4 * hh:4 * hh + 4]
                    qs = ps_stc[:, 8 + 4 * hh:8 + 4 * hh + 4]
                    nc.vector.tensor_scalar(out=errc[:], in0=ks,
                                            scalar1=abc[:, hh:hh + 1],
                                            scalar2=None, op0=OP.mult)
                    nc.vector.tensor_sub(errc[:], v1c[:, 4 * hh:4 * hh + 4], errc[:])
                    nc.vector.tensor_scalar(out=t1c[:], in0=qs,
                                            scalar1=abc[:, hh:hh + 1],
                                            scalar2=None, op0=OP.mult)
                    nc.vector.tensor_scalar(out=errc[:], in0=errc[:],
                                            scalar1=abc[:, 2 + hh:3 + hh],
                                            scalar2=None, op0=OP.mult)
                    nc.vector.tensor_add(ovc[:, 4 * hh:4 * hh + 4], t1c[:], errc[:])

                # ---- ov to bf16 columns ----
                ov_b = sm.tile([128, 8], BF, tag="ov_b")
                nc.vector.tensor_copy(ov_b[:], ovc[:])

                # ---- output projection ----
                ps_o = [pm.tile([1, 512], F32, tag="ps", name=f"ps_o{i}")
                        for i in range(8)]
                out_sb = sm.tile([1, H], F32, tag="out_sb")
                for dd in range(3):
                    t = tiles[8 + dd]
                    for i in range(2):
                        j = 2 * dd + i
                        for it in range(8):
                            nc.tensor.matmul(
                                ps_o[it][0:1, :], ov_b[:, j:j + 1],
                                t[:, 4096 * i + 512 * it:4096 * i + 512 * it + 512],
                                start=(j == 0), stop=False)
                t = tiles[11]  # j = 6 (1MB)
                for it in range(8):
                    nc.tensor.matmul(
                        ps_o[it][0:1, :], ov_b[:, 6:7],
                        t[:, 512 * it:512 * it + 512], start=False, stop=False)
                t = tiles[12]  # j = 7 (1MB); copy each strip as it closes
                for it in range(8):
                    nc.tensor.matmul(
                        ps_o[it][0:1, :], ov_b[:, 7:8],
                        t[:, 512 * it:512 * it + 512], start=False, stop=True)
                    dst = out_sb[0:1, 512 * it:512 * it + 512]
                    if it % 2 == 0:
                        nc.vector.tensor_copy(dst, ps_o[it][0:1, :])
                    else:
                        nc.scalar.copy(dst, ps_o[it][0:1, :])
                nc.sync.dma_start(out=out_d[:], in_=out_sb[:])

            emit()

    nc.finalize()
    return nc


def _prep_in_maps(inputs):
    f32 = np.float32
    hid = np.asarray(inputs["hidden_states"], f32)[0, :, 0, 0]     # [4096]
    Wq = np.asarray(inputs["Wq"], f32)
    Wk = np.asarray(inputs["Wk"], f32)
    Wv = np.asarray(inputs["Wv"], f32)
    Wo = np.asarray(inputs["Wo"], f32)
    Wa = np.asarray(inputs["Wa"], f32)
    Wb = np.asarray(inputs["Wb"], f32)
    qcw = np.asarray(inputs["q_conv_w"], f32)[0]                   # [QK, 4]
    kcw = np.asarray(inputs["k_conv_w"], f32)[0]
    vcw = np.asarray(inputs["v_conv_w"], f32)[0]                   # [VD, 4]
    qca = np.asarray(inputs["q_cache"], f32)[0]                    # [QK, 3]
    kca = np.asarray(inputs["k_cache"], f32)[0]
    vca = np.asarray(inputs["v_cache"], f32)[0]                    # [VD, 3]
    state = np.asarray(inputs["state"], f32)[0]                    # [16,256,512]

    # h in column layout [128, 32], single bf16
    hb_c = np.ascontiguousarray(hid.reshape(32, 128).T.astype(BF16))

    in_maps = []
    for c in range(NCORES):
        rq = slice(c * RQ, (c + 1) * RQ)
        rv = slice(c * RV, (c + 1) * RV)
        # packed [Wq ; Wk] rows -> transposed [H, 1024] bf16
        wqkt = np.concatenate([Wq[rq], Wk[rq]], axis=0).T.astype(BF16)
        img_qk = wqkt.reshape(4, 8, 128, 1024).transpose(0, 2, 1, 3)
        wvt = Wv[rv].T.astype(BF16)                                # [H, 1024]
        img_v = wvt.reshape(4, 8, 128, 1024).transpose(0, 2, 1, 3)
        wot = Wo[:, rv].T.astype(BF16)                             # [1024, H]
        img_o = wot.reshape(4, 2, 128, 4096).transpose(0, 2, 1, 3)
        wbig = np.concatenate([img_qk.reshape(512, 8192),
                               img_v.reshape(512, 8192),
                               img_o.reshape(512, 8192)], axis=0)

        wab = np.concatenate([Wa[2 * c:2 * c + 2], Wb[2 * c:2 * c + 2]], 0)
        wab_sb = np.ascontiguousarray(
            wab.reshape(4, 32, 128).transpose(2, 1, 0).reshape(128, 128)
            .astype(BF16))
        st_sb = np.ascontiguousarray(
            state[2 * c:2 * c + 2].reshape(2, 2, 128, 512)
            .transpose(2, 0, 1, 3).reshape(128, 2048).astype(BF16))

        # q/k conv in column layout [128, 8*taps]: per tap, cols 0-3 = k
        # chunks (k idx 128c+p), cols 4-7 = q chunks
        qk_ca = np.concatenate(
            [np.concatenate([kca[rq, t].reshape(4, 128).T,
                             qca[rq, t].reshape(4, 128).T], 1)
             for t in range(3)], 1)
        qk_cw = np.concatenate(
            [np.concatenate([kcw[rq, t].reshape(4, 128).T,
                             qcw[rq, t].reshape(4, 128).T], 1)
             for t in range(4)], 1)
        # v conv in column layout [128, 8*taps]: vcol[p, 8t+cc] = v[128cc+p, t]
        v_ca = np.ascontiguousarray(
            vca[rv].reshape(8, 128, 3).transpose(1, 2, 0).reshape(128, 24))
        v_cw = np.ascontiguousarray(
            vcw[rv].reshape(8, 128, 4).transpose(1, 2, 0).reshape(128, 32))

        in_maps.append({
            "wbig": np.ascontiguousarray(wbig),
            "hb": hb_c, "wabt": wab_sb, "state_c": st_sb,
            "qkcache": np.ascontiguousarray(qk_ca),
            "qkconvw": np.ascontiguousarray(qk_cw),
            "vcache": v_ca, "vconvw": v_cw,
        })
    return in_maps


def _run(inputs, trace=False, tmpdir=None):
    _ensure_ntff_hook()
    if "nc" not in _CACHE:
        _CACHE["nc"] = _build_nc()
    nc = _CACHE["nc"]
    in_maps = _prep_in_maps(inputs)
    res = run_bass_kernel_spmd(nc, in_maps, list(range(NCORES)),
                               trace=trace, tmpdir=tmpdir)
    acc = np.zeros(H, np.float64)
    for c in range(NCORES):
        acc += res.results[c]["out"][0].astype(np.float64)
    out = acc.astype(np.float32).reshape(1, H, 1, 1)
    return out, res


def kernel(**inputs):
    out, _ = _run(inputs, trace=False)
    return out


def kernel_traced(tmpdir=None, **inputs):
    return _run(inputs, trace=True, tmpdir=tmpdir)
